# revision 8
# baseline (speedup 1.0000x reference)
"""Causal self-attention on 8 trn2 NeuronCores.

Full inputs in, full output out. Sharding: data-parallel over batch (B=4),
tensor-parallel over head groups (16 heads -> 2 groups of 8). core = 2*b + g.

Per-core math (T=2048, C=1024, 8 heads, D=64, group channels G=512):
  qT/kT: [64*(h%2)+d, h//2, t] layout so scores need no transposes
  scoresT[j,i] = sum_d kT[d,j] qT[d,i]   (q pre-scaled by 1/sqrt(D) on host)
  softmax without max-subtraction (scores ~ N(0,1); exp is shift-invariant)
  expT row sums ride free on a 65th all-ones column appended to V
  causal mask via affine_select (fill 0 post-exp) on diagonal blocks only
  out_T[d,i] = sum_j v[j,d] expT[j,i]; normalize by 1/sums; y = oT.T @ Wp

Schedule (v2): every matmul is chained with an order-only dep so the PE
executes exactly the emission order. Emission interleaves the attention
blocks (scores -> exp on ACT -> A@V) with a background queue of
independent matmul work (v projection tiles, q/k projection groups for
later head-pairs) so the PE never waits on the scalar engine's exp.
q/k and output projections reuse the stationary operand across two
matmuls via ldweights=False (saves ~40ns/matmul of LDWEIGHTS time).

Host gather: y[b] = part[2b] + part[2b+1] + b_attn_v @ W_proj + b_proj
(q/k biases are added on-device; the v bias commutes through softmax).
"""

import numpy as np
from collections import deque
from contextlib import ExitStack

import concourse.bass as bass
import concourse.tile as tile
from concourse import bacc, mybir
from concourse.bass_utils import run_bass_kernel_spmd
from concourse.tile import add_dep_helper

P = 128
B, T, C, H = 4, 2048, 1024, 16
D = 64
HG = 8          # heads per core
G = HG * D      # 512 head channels per core
CT = C // P     # 8 contraction tiles
TCH = T // 512  # 4 chunks of 512 tokens
NT = G // P     # 4 tiles of head channels (also: head-pair index)
NTT = T // P    # 16 token tiles of 128

f32 = mybir.dt.float32
bf16 = mybir.dt.bfloat16
MM_DT = bf16
EXP_DT = MM_DT


def build_attention(nc: bass.Bass):
    xT = nc.dram_tensor("xT", [C, T], MM_DT, kind="ExternalInput")
    wq = nc.dram_tensor("wq", [C, G], MM_DT, kind="ExternalInput")
    wk = nc.dram_tensor("wk", [C, G], MM_DT, kind="ExternalInput")
    wv = nc.dram_tensor("wv", [C, G], MM_DT, kind="ExternalInput")
    wp = nc.dram_tensor("wp", [G, C], MM_DT, kind="ExternalInput")
    bq = nc.dram_tensor("bq", [P, NT], f32, kind="ExternalInput")
    bk = nc.dram_tensor("bk", [P, NT], f32, kind="ExternalInput")
    y = nc.dram_tensor("y", [T, C], f32, kind="ExternalOutput")

    with tile.TileContext(nc) as tc, ExitStack() as ctx:
        persist = ctx.enter_context(tc.tile_pool(name="persist", bufs=1))
        qT = persist.tile([P, NT, T], MM_DT)
        kT = persist.tile([P, NT, T], MM_DT)
        v_aug = persist.tile([P, NTT, HG, D + 1], MM_DT)
        x_sb = persist.tile([P, CT, T], MM_DT)
        wq_sb = persist.tile([P, CT, G], MM_DT)
        wk_sb = persist.tile([P, CT, G], MM_DT)
        wv_sb = persist.tile([P, CT, G], MM_DT)
        wp_sb = persist.tile([P, NT, C], MM_DT)
        bq_sb = persist.tile([P, NT], f32)
        bk_sb = persist.tile([P, NT], f32)
        oT = persist.tile([P, NT, T], MM_DT)

        # DMA order matters: first x + wv (v units start first), then wq/wk,
        # then wp (needed last).
        for ct in range(CT):
            nc.sync.dma_start(out=x_sb[:, ct, :], in_=xT.ap()[P * ct:P * (ct + 1), :])
            nc.sync.dma_start(out=wv_sb[:, ct, :], in_=wv.ap()[P * ct:P * (ct + 1), :])
        for ct in range(CT):
            nc.sync.dma_start(out=wq_sb[:, ct, :], in_=wq.ap()[P * ct:P * (ct + 1), :])
            nc.sync.dma_start(out=wk_sb[:, ct, :], in_=wk.ap()[P * ct:P * (ct + 1), :])
        nc.sync.dma_start(out=bq_sb, in_=bq.ap())
        nc.sync.dma_start(out=bk_sb, in_=bk.ap())
        for nt in range(NT):
            nc.sync.dma_start(out=wp_sb[:, nt, :], in_=wp.ap()[P * nt:P * (nt + 1), :])

        ones_col = persist.tile([P, 1], f32)
        nc.vector.memset(ones_col, 1.0)
        nc.vector.tensor_copy(
            out=v_aug[:, :, :, D:D + 1],
            in_=ones_col.to_broadcast([P, NTT, HG, 1]),
        )

        # ---- global PE order chain ----
        last_mm = [None]

        def MM(out, lhsT, rhs, start, stop, reuse=False):
            mm = nc.tensor.matmul(out, lhsT, rhs, start=start, stop=stop)
            if reuse:
                mm.ins.ldweights = False
            if last_mm[0] is not None:
                add_dep_helper(mm.ins, last_mm[0].ins, sync=False, reason="pe-order")
            last_mm[0] = mm
            return mm

        with (
            tc.tile_pool(name="ps_bg", bufs=2, space="PSUM") as ps_bg,
            tc.tile_pool(name="ps_o", bufs=2, space="PSUM") as opool,
            tc.tile_pool(name="ps_s", bufs=2, space="PSUM") as spool,
            tc.tile_pool(name="epool", bufs=6) as epool,
            tc.tile_pool(name="upool", bufs=10) as upool,
            tc.tile_pool(name="srpool", bufs=2) as srpool,
            tc.tile_pool(name="bpool", bufs=3) as bpool,
        ):
            # ---------- background unit generators ----------
            def v_gen(tt):
                ps = ps_bg.tile([P, G], f32, tag="bg", name=f"pv_{tt}")
                for ct in range(CT):
                    MM(ps, x_sb[:, ct, P * tt:P * (tt + 1)],
                       wv_sb[:, ct, :], start=(ct == 0), stop=(ct == CT - 1))
                    if ct == 3:
                        yield
                nc.vector.tensor_copy(
                    out=v_aug[:, tt, :, 0:D],
                    in_=ps.rearrange("p (h d) -> p h d", h=HG))
                yield

            def kq_gen(w_sb, b_sb, dstT, jt, th):
                """One (weight, head-pair, tch-half) group: 8 units of
                LDW + 2 MMs (ldweights reuse across the two token chunks)."""
                pa = ps_bg.tile([P, 512], f32, tag="bg",
                                name=f"pkq_{id(w_sb)}_{jt}_{th}_a")
                pb = ps_bg.tile([P, 512], f32, tag="bg",
                                name=f"pkq_{id(w_sb)}_{jt}_{th}_b")
                base = 1024 * th
                for ct in range(CT):
                    MM(pa, w_sb[:, ct, P * jt:P * (jt + 1)],
                       x_sb[:, ct, base:base + 512],
                       start=(ct == 0), stop=(ct == CT - 1))
                    MM(pb, w_sb[:, ct, P * jt:P * (jt + 1)],
                       x_sb[:, ct, base + 512:base + 1024],
                       start=(ct == 0), stop=(ct == CT - 1), reuse=True)
                    if ct < CT - 1:
                        yield
                nc.vector.tensor_scalar_add(
                    out=dstT[:, jt, base:base + 512], in0=pa,
                    scalar1=b_sb[:, jt:jt + 1])
                nc.vector.tensor_scalar_add(
                    out=dstT[:, jt, base + 512:base + 1024], in0=pb,
                    scalar1=b_sb[:, jt:jt + 1])
                yield

            def drain(gen):
                for _ in gen:
                    pass

            # background queue entries: (kind, key, gen); v tiles 4..15 first
            # (needed by pair0's A@V), then k/q groups for pairs 1..3.
            bg = deque()
            for tt in range(4, NTT):
                bg.append(("v", tt, v_gen(tt)))
            for g2 in range(1, NT):
                for th in range(2):
                    bg.append(("kq", g2, kq_gen(wk_sb, bk_sb, kT, g2, th)))
                    bg.append(("kq", g2, kq_gen(wq_sb, bq_sb, qT, g2, th)))

            bg_units_total = 12 * 2 + 12 * 8  # v half-units + kq ct-units
            pulled = [0]

            def pull_bg(n=1):
                for _ in range(n):
                    if not bg:
                        return
                    try:
                        next(bg[0][2])
                        pulled[0] += 1
                    except StopIteration:
                        bg.popleft()

            def force_drain(pred):
                """Advance the queue front until no entry matches pred."""
                while any(pred(kind, key) for kind, key, _ in bg):
                    pull_bg(1)

            # ---------- prologue: v tiles 0..3, then k/q for pair 0 ----
            for tt in range(4):
                drain(v_gen(tt))
            for th in range(2):
                drain(kq_gen(wk_sb, bk_sb, kT, 0, th))
                drain(kq_gen(wq_sb, bq_sb, qT, 0, th))

            # ---------- attention, pair-major, pipelined depth 2 --------
            n_blocks_total = 160
            done_blocks = [0]

            def bg_target(done):
                pts = [(0, 0), (40, 56), (80, 88), (112, 120), (160, 120)]
                for (d0, t0), (d1, t1) in zip(pts, pts[1:]):
                    if done <= d1:
                        return t0 + (t1 - t0) * (done - d0) // max(1, d1 - d0)
                return bg_units_total

            def pace():
                want = bg_target(done_blocks[0] + 1) - pulled[0]
                if want > 0:
                    pull_bg(min(want, 3))

            for g2 in range(NT):
                if g2 > 0:
                    force_drain(lambda kind, key: kind == "kq" and key <= g2)
                blocks = []
                for ic in range(TCH):
                    for jb in range(4 * ic + 4):
                        blocks.append((ic, jb))

                s_tiles = {}
                e_tiles = {}

                def emit_scores(b):
                    ic, jb = blocks[b]
                    off = max(0, P * jb - 512 * ic)
                    s_big = spool.tile([P, 1024], f32, tag="s",
                                       name=f"sps_{g2}_{ic}_{jb}")
                    for hh in range(2):
                        band = 64 * hh
                        MM(s_big[:, 512 * hh + off:512 * (hh + 1)],
                           kT[band:band + D, g2, P * jb:P * (jb + 1)],
                           qT[band:band + D, g2, 512 * ic + off:512 * (ic + 1)],
                           start=True, stop=True)
                    e_big = epool.tile([P, 2, 512], EXP_DT, tag="e",
                                       name=f"e_{g2}_{ic}_{jb}")
                    nc.scalar.activation(
                        out=e_big[:, :, off:],
                        in_=s_big.rearrange("p (h2 i) -> p h2 i", h2=2)[:, :, off:],
                        func=mybir.ActivationFunctionType.Exp,
                    )
                    if P * jb >= 512 * ic:
                        for hh in range(2):
                            nc.gpsimd.affine_select(
                                out=e_big[:, hh, off:off + P],
                                in_=e_big[:, hh, off:off + P],
                                compare_op=mybir.AluOpType.is_ge,
                                fill=0.0, base=0, channel_multiplier=-1,
                                pattern=[[1, P]],
                            )
                    e_tiles[b] = e_big

                o_ps = {}
                o_us = {}

                def emit_av(b):
                    ic, jb = blocks[b]
                    off = max(0, P * jb - 512 * ic)
                    if jb >= 4:
                        force_drain(lambda kind, key: kind == "v" and key <= jb)
                    if jb == 0:
                        for hh in range(2):
                            o_ps[hh] = opool.tile([D + 1, 512], f32, tag="o",
                                                  name=f"ops_{2 * g2 + hh}_{ic}")
                    e_big = e_tiles.pop(b)
                    n_jb = 4 * ic + 4
                    for hh in range(2):
                        h = 2 * g2 + hh
                        MM(o_ps[hh][:, off:], v_aug[:, jb, h, :],
                           e_big[:, hh, off:],
                           start=(jb == 0), stop=(jb == n_jb - 1))
                    if jb == n_jb - 1:
                        for hh in range(2):
                            h = 2 * g2 + hh
                            idx = 4 * hh + ic
                            o_u = upool.tile([D + 1, 512], f32, tag="ou",
                                             name=f"ou_{h}_{ic}")
                            nc.vector.tensor_copy(o_u, o_ps[hh])
                            nc.sync.dma_start(out=S_pair[idx:idx + 1, :],
                                              in_=o_u[D:D + 1, :])
                            o_us[idx] = o_u

                S_pair = srpool.tile([8, 512], f32, tag="S", name=f"S_{g2}")
                emit_scores(0)
                pace()
                for b in range(len(blocks)):
                    if b + 1 < len(blocks):
                        emit_scores(b + 1)
                    pace()
                    emit_av(b)
                    done_blocks[0] += 1

                # normalization for this pair (off the PE chain); ic-major
                # so the projection's low token tiles unblock first
                R_pair = srpool.tile([8, 512], f32, tag="R", name=f"R_{g2}")
                nc.vector.reciprocal(R_pair, S_pair)
                for ic in range(TCH):
                    for hh in range(2):
                        h = 2 * g2 + hh
                        idx = 4 * hh + ic
                        rrow = bpool.tile([1, 512], f32, tag="rrow",
                                          name=f"rr_{h}_{ic}")
                        nc.sync.dma_start(out=rrow, in_=R_pair[idx:idx + 1, :])
                        rb = bpool.tile([D, 512], f32, tag="rb", name=f"rb_{h}_{ic}")
                        nc.gpsimd.partition_broadcast(rb, rrow[0:1, :])
                        nc.vector.tensor_mul(
                            out=oT[64 * (h % 2):64 * (h % 2) + D, h // 2,
                                   512 * ic:512 * (ic + 1)],
                            in0=o_us[idx][0:D, :],
                            in1=rb,
                        )

            # drain any remaining background work (shouldn't be much)
            while bg:
                pull_bg(1)

        # ---------- output projection, tt-major with mc ldweights reuse ----
        with (
            tc.tile_pool(name="ypool", bufs=4) as ypool,
            tc.tile_pool(name="ps_y", bufs=4, space="PSUM") as ps_y,
        ):
            for tt in range(NTT):
                yp = [ps_y.tile([P, 512], f32, tag="y", name=f"y_{tt}_{mc}")
                      for mc in range(2)]
                for nt in range(NT):
                    MM(yp[0], oT[:, nt, P * tt:P * (tt + 1)],
                       wp_sb[:, nt, 0:512],
                       start=(nt == 0), stop=(nt == NT - 1))
                    MM(yp[1], oT[:, nt, P * tt:P * (tt + 1)],
                       wp_sb[:, nt, 512:1024],
                       start=(nt == 0), stop=(nt == NT - 1), reuse=True)
                for mc in range(2):
                    y_sb = ypool.tile([P, 512], f32, tag="ysb",
                                      name=f"ysb_{tt}_{mc}")
                    nc.vector.tensor_copy(out=y_sb, in_=yp[mc])
                    nc.sync.dma_start(
                        out=y.ap()[P * tt:P * (tt + 1), 512 * mc:512 * (mc + 1)],
                        in_=y_sb,
                    )


_NC_CACHE = {}


def _get_nc():
    if "nc" not in _NC_CACHE:
        nc = bacc.Bacc("TRN2", debug=False, num_devices=8)
        build_attention(nc)
        nc.compile()
        _NC_CACHE["nc"] = nc
    return _NC_CACHE["nc"]


def kernel(x, W_attn, b_attn, W_proj, b_proj):
    x = np.asarray(x, dtype=np.float32)
    W_attn = np.asarray(W_attn, dtype=np.float32)
    b_attn = np.asarray(b_attn, dtype=np.float32)
    W_proj = np.asarray(W_proj, dtype=np.float32)
    b_proj = np.asarray(b_proj, dtype=np.float32)

    import ml_dtypes
    mm_np = ml_dtypes.bfloat16

    scale = 1.0 / np.sqrt(np.float32(D))
    in_maps = []
    for core in range(8):
        b, g = divmod(core, 2)
        cols = slice(G * g, G * (g + 1))
        bqs = (b_attn[0:C][cols] * scale).reshape(NT, 2, D).transpose(1, 2, 0).reshape(P, NT)
        bks = b_attn[C:2 * C][cols].reshape(NT, 2, D).transpose(1, 2, 0).reshape(P, NT)
        in_maps.append({
            "xT": np.ascontiguousarray(x[b].T).astype(mm_np),
            "wq": np.ascontiguousarray(W_attn[:, 0:C][:, cols] * scale).astype(mm_np),
            "wk": np.ascontiguousarray(W_attn[:, C:2 * C][:, cols]).astype(mm_np),
            "wv": np.ascontiguousarray(W_attn[:, 2 * C:3 * C][:, cols]).astype(mm_np),
            "wp": np.ascontiguousarray(W_proj[G * g:G * (g + 1), :]).astype(mm_np),
            "bq": np.ascontiguousarray(bqs),
            "bk": np.ascontiguousarray(bks),
        })

    res = run_bass_kernel_spmd(_get_nc(), in_maps, core_ids=list(range(8)))

    correction = b_attn[2 * C:3 * C] @ W_proj + b_proj  # [C]
    out = np.empty((B, T, C), dtype=np.float32)
    for b in range(B):
        out[b] = res.results[2 * b]["y"] + res.results[2 * b + 1]["y"] + correction
    return out
